# revision 47
# baseline (speedup 1.0000x reference)
"""CrossAttention Trainium2 kernel.

Sharding: 8 cores = 4 batches x 2 head-groups (8 heads each).
Per core: q/k/v projections for its 512-dim head slice, per-head
attention (scores^T orientation, ones-column denominator), out
projection against the matching 512-row slice of wo. Host sums the
two head-group partials per batch and adds bo (+ bv @ wo, folded out
of the kernel: softmax rows sum to 1, so the v-bias passes through
attention unchanged; the k-bias shifts every score for a query by the
same amount and cancels in softmax, so it is dropped entirely).

Matmuls run in bf16 (fp32 PSUM accumulation). Attention processes q
in blocks of 512; heads run in pairs (2d, 2d+1) whose score matmuls
occupy disjoint 64-row halves of the PE array (tile_position row
groups) and execute concurrently. Each pair's two score tiles share
one [128, 2, 512] PSUM tile so a single N=1024 ACTIVATE(exp) covers
both heads.
"""

import numpy as np
from contextlib import ExitStack

import concourse.bass as bass
from concourse import bacc
import concourse.tile as tile
import concourse.mybir as mybir
from concourse.bass_utils import run_bass_kernel_spmd

F32 = mybir.dt.float32
BF16 = mybir.dt.bfloat16

S = 2048          # sequence length
D = 1024          # d_model
DS = 512          # per-core head-slice width (8 heads x 64)
H = 8             # heads per core
DH = 64           # head dim
KC = D // 128     # 8 contraction chunks of 128 for the qkv projections
QB = 512          # attention q-block width
NQB = S // QB     # 4 q-blocks
NKI = S // 128    # 16 key tiles of 128


def build_nc():
    nc = bacc.Bacc("TRN2")

    # inputs arrive pre-slabbed from the host so every DMA line is a long
    # contiguous run (8KB/partition) instead of 1KB strided packets
    xs = nc.declare_dram_parameter("xs", [4, 128, KC, 512], BF16, isOutput=False)
    ys = nc.declare_dram_parameter("ys", [4, 128, KC, 512], BF16, isOutput=False)
    wq = nc.declare_dram_parameter("wq", [128, KC, DS], BF16, isOutput=False)
    wk = nc.declare_dram_parameter("wk", [128, KC, DS], BF16, isOutput=False)
    wv = nc.declare_dram_parameter("wv", [128, KC, DS], BF16, isOutput=False)
    bq = nc.declare_dram_parameter("bq", [128, 4], F32, isOutput=False)
    wo = nc.declare_dram_parameter("wo", [128, 4, D], BF16, isOutput=False)
    out = nc.declare_dram_parameter("out", [S, D], F32, isOutput=True)

    with tile.TileContext(nc) as tc, ExitStack() as ctx:
        # ---- persistent SBUF pools --------------------------------------
        kv_pool = ctx.enter_context(tc.tile_pool(name="kv", bufs=1))
        w_pool = ctx.enter_context(tc.tile_pool(name="wp", bufs=1))
        qt_pool = ctx.enter_context(tc.tile_pool(name="qt", bufs=1))
        att_pool = ctx.enter_context(tc.tile_pool(name="att", bufs=1))
        const_pool = ctx.enter_context(tc.tile_pool(name="const", bufs=1))
        y_pool = ctx.enter_context(tc.tile_pool(name="ysl", bufs=4))
        x_pool = ctx.enter_context(tc.tile_pool(name="xsl", bufs=3))
        pt_pool = ctx.enter_context(tc.tile_pool(name="pt", bufs=12))
        dn_pool = ctx.enter_context(tc.tile_pool(name="dn", bufs=4))
        bcs_pool = ctx.enter_context(tc.tile_pool(name="bcs", bufs=4))
        ost_pool = ctx.enter_context(tc.tile_pool(name="ost", bufs=3))
        # ---- PSUM pools: 4 + 2 + 2 = 8 banks ----------------------------
        psc = ctx.enter_context(tc.tile_pool(name="psc", bufs=2, space="PSUM"))
        pav = ctx.enter_context(tc.tile_pool(name="pav", bufs=2, space="PSUM"))
        psmall = ctx.enter_context(tc.tile_pool(name="psmall", bufs=2, space="PSUM"))

        # kT/qT: [d, s] per d-block; d-block d holds heads 2d (parts 0:64)
        # and 2d+1 (parts 64:128)
        kT = [kv_pool.tile([128, S], BF16, tag=f"kT{d}", name=f"kT{d}") for d in range(4)]
        qT = [qt_pool.tile([128, S], BF16, tag=f"qT{d}", name=f"qT{d}") for d in range(4)]
        attnT = [att_pool.tile([128, S], BF16, tag=f"attnT{d}", name=f"attnT{d}") for d in range(4)]
        # v: s-tiles [128, 8, 65] -- per head 64 v-cols + 1 ones-col (denominator)
        v_sb = [kv_pool.tile([128, H, DH + 1], BF16, tag=f"v{i}", name=f"v{i}") for i in range(NKI)]

        wq_sb = w_pool.tile([128, KC, DS], BF16, tag="wq")
        wk_sb = w_pool.tile([128, KC, DS], BF16, tag="wk")
        wv_sb = w_pool.tile([128, KC, DS], BF16, tag="wv")
        wo_sb = w_pool.tile([128, 4, D], BF16, tag="wo")
        bq_sb = const_pool.tile([128, 4], F32, tag="bq")
        ones_b = const_pool.tile([1, DH], BF16, tag="ones_b")

        nc.gpsimd.memset(ones_b[:], 1.0)
        for i in range(NKI):
            nc.gpsimd.memset(v_sb[i][:, :, DH], 1.0)

        # ---- input DMAs (prefetch order matters for the head phase) -----
        nc.sync.dma_start(out=wk_sb[:], in_=wk[:])
        y_slab = [y_pool.tile([128, KC, 512], BF16, tag="yslab", name=f"ysl{s}")
                  for s in range(4)]
        x_slab = [x_pool.tile([128, KC, 512], BF16, tag="xslab", name=f"xsl{s}")
                  for s in range(4)]
        nc.sync.dma_start(out=y_slab[0][:], in_=ys[0])
        nc.sync.dma_start(out=wq_sb[:], in_=wq[:])
        nc.sync.dma_start(out=x_slab[0][:], in_=xs[0])
        nc.sync.dma_start(out=bq_sb[:], in_=bq[:])
        for s in range(1, 4):
            nc.sync.dma_start(out=y_slab[s][:], in_=ys[s])
        nc.sync.dma_start(out=wv_sb[:], in_=wv[:])
        nc.sync.dma_start(out=wo_sb[:], in_=wo[:])
        for s in range(1, 4):
            nc.sync.dma_start(out=x_slab[s][:], in_=xs[s])

        # ---- projection helpers (psmall groups: 1 bank each) ------------
        # Each projection chunk is split into two half-groups (4 contraction
        # steps each) so filler pieces slot into the exp stream's PE idle
        # without long FIFO runs that stall the next score pair.
        def proj_pieces(name, ps_tag, lhs_fn, rhs_fn, fin, npiece=4):
            # allocate the PSUM tile lazily at first emission so the pool's
            # buffer-rotation order matches the actual write order; 2-MM
            # pieces fit the PE slack under one exp without starving it
            state = {}
            step = KC // npiece

            def piece(h):
                if h == 0:
                    state["ps"] = psmall.tile([128, 512], F32, tag="psmall", name=ps_tag)
                ps = state["ps"]
                for k0 in range(h * step, (h + 1) * step):
                    nc.tensor.matmul(
                        out=ps[:], lhsT=lhs_fn(k0), rhs=rhs_fn(k0),
                        start=(k0 == 0), stop=(k0 == KC - 1),
                    )
                if h == npiece - 1:
                    fin(ps)
            return piece

        def k_proj_pieces(d, sb, npiece=4):
            return proj_pieces(
                f"k{d}_{sb}", f"psk{d}_{sb}",
                lambda k0: wk_sb[:, k0, d * 128:(d + 1) * 128],
                lambda k0: y_slab[sb][:, k0, :],
                lambda ps: nc.vector.tensor_copy(
                    out=kT[d][:, sb * 512:(sb + 1) * 512], in_=ps[:]),
                npiece,
            )

        def q_proj_pieces(d, sb, npiece=4):
            return proj_pieces(
                f"q{d}_{sb}", f"psq{d}_{sb}",
                lambda k0: wq_sb[:, k0, d * 128:(d + 1) * 128],
                lambda k0: x_slab[sb][:, k0, :],
                lambda ps: nc.vector.tensor_scalar_add(
                    out=qT[d][:, sb * 512:(sb + 1) * 512],
                    in0=ps[:], scalar1=bq_sb[:, d:d + 1]),
                npiece,
            )

        def v_proj_pieces(sb, st, npiece=4):
            def fin(ps):
                vt = v_sb[sb * 4 + st]
                nc.vector.tensor_copy(
                    out=vt[:, :, 0:DH],
                    in_=ps[:].rearrange("p (h e) -> p h e", h=H),
                )
            return proj_pieces(
                f"v{sb}_{st}", f"psv{sb}_{st}",
                lambda k0: y_slab[sb][:, k0, st * 128:(st + 1) * 128],
                lambda k0: wv_sb[:, k0, :],
                fin,
                npiece,
            )

        def k_proj(d, sb):
            p = k_proj_pieces(d, sb, 1)
            p(0)

        def q_proj(d, sb):
            p = q_proj_pieces(d, sb, 1)
            p(0)

        def v_proj(sb, st):
            p = v_proj_pieces(sb, st, 1)
            p(0)

        def out_proj_pieces(qb, qt, nb):
            # out rows q0..q0+128, cols nb*512..: attnT^T @ wo, in two
            # 2-MM pieces (lazy PSUM alloc at first piece)
            q0 = qb * QB + qt * 128
            state = {}

            def piece(h):
                if h == 0:
                    state["ps"] = psmall.tile(
                        [128, 512], F32, tag="psmall", name=f"pso{qb}_{qt}_{nb}")
                ps = state["ps"]
                for d in (2 * h, 2 * h + 1):
                    nc.tensor.matmul(
                        out=ps[:],
                        lhsT=attnT[d][:, q0:q0 + 128],
                        rhs=wo_sb[:, d, nb * 512:(nb + 1) * 512],
                        start=(d == 0), stop=(d == 3),
                    )
                if h == 1:
                    ost = ost_pool.tile([128, 512], F32, tag="ost", name=f"ost{qb}_{qt}_{nb}")
                    nc.vector.tensor_copy(out=ost[:], in_=ps[:])
                    nc.sync.dma_start(
                        out=out[q0:q0 + 128, nb * 512:(nb + 1) * 512], in_=ost[:])
            return piece

        def out_proj_half(qb, qt, nb):
            p = out_proj_pieces(qb, qt, nb)
            p(0)
            p(1)

        def out_proj(qb, qt):
            out_proj_half(qb, qt, 0)
            out_proj_half(qb, qt, 1)

        # ---- one attention round: head pair d, q-block qb ---------------
        # Emission order ~= engine FIFO order, so: the sc-pair for ki+1 and
        # exp(ki+1) are emitted BEFORE av(ki) (stagger) to keep the exp
        # stream free of the av dependency; filler work (projections, norms,
        # out-proj chunks) is spliced into per-ki slots so it lands in the
        # PE idle time under the ACT-bound exp stream.
        def round_ki(d, qb, fillers=None):
            qsl = slice(qb * QB, (qb + 1) * QB)
            av0 = pav.tile([DH + 1, QB], F32, tag="pav", name=f"av0_{d}_{qb}")
            av1 = pav.tile([DH + 1, QB], F32, tag="pav", name=f"av1_{d}_{qb}")

            def av_pair(ki):
                nc.tensor.matmul(
                    out=av0[:], lhsT=v_sb[ki][:, 2 * d, :], rhs=pt2s[ki][:, 0, :],
                    start=(ki == 0), stop=(ki == NKI - 1),
                )
                nc.tensor.matmul(
                    out=av1[:], lhsT=v_sb[ki][:, 2 * d + 1, :], rhs=pt2s[ki][:, 1, :],
                    start=(ki == 0), stop=(ki == NKI - 1),
                )

            pt2s = {}
            for ki in range(NKI):
                ksl = slice(ki * 128, (ki + 1) * 128)
                sc2 = psc.tile([128, 2, QB], F32, tag="psc", name=f"sc{d}_{qb}_{ki}")
                nc.tensor.matmul(
                    out=sc2[:, 0, :], lhsT=kT[d][0:DH, ksl], rhs=qT[d][0:DH, qsl],
                    start=True, stop=True,
                )
                nc.tensor.matmul(
                    out=sc2[:, 1, :], lhsT=kT[d][DH:128, ksl], rhs=qT[d][DH:128, qsl],
                    start=True, stop=True,
                )
                pt2 = pt_pool.tile([128, 2, QB], BF16, tag="pt", name=f"pt{d}_{qb}_{ki}")
                pt2s[ki] = pt2
                nc.scalar.activation(
                    out=pt2[:], in_=sc2[:],
                    func=mybir.ActivationFunctionType.Exp, scale=0.125,
                )
                if fillers and ki in fillers:
                    for f in fillers[ki]:
                        f()
                if ki >= 1:
                    av_pair(ki - 1)
            # av_pair(15) is deferred: the caller emits it inside the NEXT
            # round (slot 0) so the next round's first score pair isn't
            # FIFO-stuck behind it at the boundary
            return av0, av1, lambda: av_pair(NKI - 1)

        # normalize: rec = 1/den (den = ones-row 64), broadcast over the
        # 64 dh partitions through the PE, multiply on the copy out
        def round_norm(d, qb, avs):
            for j, av in enumerate(avs):
                poff = DH * j
                asl = attnT[d][poff:poff + DH, qb * QB:(qb + 1) * QB]
                nc.vector.tensor_copy(out=asl, in_=av[0:DH, :])
                den = dn_pool.tile([1, QB], F32, tag="den", name=f"den{d}_{qb}_{j}")
                nc.vector.tensor_copy(out=den[:], in_=av[DH:DH + 1, :])
                rec = dn_pool.tile([1, QB], F32, tag="rec", name=f"rec{d}_{qb}_{j}")
                nc.vector.reciprocal_approx_fast(out=rec[:], in_=den[:])
                rec16 = dn_pool.tile([1, QB], BF16, tag="rec16", name=f"rec16{d}_{qb}_{j}")
                nc.vector.tensor_copy(out=rec16[:], in_=rec[:])
                bc = psmall.tile([128, 512], F32, tag="psmall", name=f"bc{d}_{qb}_{j}")
                nc.tensor.matmul(
                    out=bc[0:DH, :], lhsT=ones_b[:], rhs=rec16[:],
                    start=True, stop=True,
                )
                nc.vector.tensor_mul(out=asl, in0=asl, in1=bc[0:DH, :])

        # ---- schedule ----------------------------------------------------
        # Head: minimum work to unblock round (d=0, qb=0): kT[d0] + qT[d0]
        # slab0. Everything else is spliced into the rounds' filler slots,
        # in an order consistent with dataflow and pool-buffer rotation.
        k_proj(0, 0)
        q_proj(0, 0)

        def F(fn, *a):
            return lambda: fn(*a)

        avs = {}

        def add(fil, ki, f):
            fil.setdefault(ki, []).append(f)

        def boundary_fillers(fil, prev):
            # previous round's deferred last av pair, then its norm
            av0p, av1p, flushp = avs.pop(prev)
            add(fil, 0, flushp)
            add(fil, 1, F(round_norm, prev[0], prev[1], (av0p, av1p)))

        # q-block-major round order: 4 pair-rounds per q-block. Fillers
        # ride as 2-MM quarters in per-ki slots: round 0 carries vproj
        # just-in-time; rounds (d,0) carry kproj(d+1) + qproj slab-0
        # pieces; later blocks carry the next q-slab and the previous
        # block's out-projection chunks.
        def add_quarters(fil, piece, ki0, stride=1):
            for j in range(4):
                add(fil, ki0 + j * stride, F(piece, j))

        for qb in range(NQB):
            for d in range(4):
                fil = {}
                if (d, qb) == (0, 0):
                    # v tiles just ahead of their avs; only kT[1] slab 0 and
                    # qT[1] slab 0 gate the next round (slabs 1-3 of kT[1]
                    # load just-in-time inside round (1,0) via subtile deps)
                    for ki in range(NKI):
                        add(fil, ki, F(v_proj, ki // 4, ki % 4))
                    # kT[0] slabs 1-3 JIT ahead of the score tiles that
                    # read them (full groups: allocation = write order)
                    add(fil, 3, F(k_proj, 0, 1))
                    add(fil, 7, F(k_proj, 0, 2))
                    add(fil, 11, F(k_proj, 0, 3))
                    add(fil, 14, F(k_proj, 1, 0))
                    add(fil, 15, F(q_proj, 1, 0))
                    avs[(d, qb)] = round_ki(d, qb, fil)
                    continue

                boundary_fillers(fil, (d - 1, qb) if d else (3, qb - 1))
                if qb == 0:
                    # rounds (1..3, 0): this pair's kT slabs 1-3 load
                    # just-in-time ahead of the score tiles that read them;
                    # the next pair's slab 0 + q-slab 0 at the end
                    add_quarters(fil, k_proj_pieces(d, 1), 0)
                    add_quarters(fil, k_proj_pieces(d, 2), 4)
                    add_quarters(fil, k_proj_pieces(d, 3), 8)
                    if d < 3:
                        add_quarters(fil, k_proj_pieces(d + 1, 0), 12)
                        add_quarters(fil, q_proj_pieces(d + 1, 0), 12)
                    else:
                        add_quarters(fil, q_proj_pieces(0, 1), 12)
                else:
                    # next pair's q-slab for this block, or next block's
                    if d < 3:
                        qh = q_proj_pieces(d + 1, qb)
                    elif qb < NQB - 1:
                        qh = q_proj_pieces(0, qb + 1)
                    else:
                        qh = None
                    if qh is not None:
                        add_quarters(fil, qh, 4)
                    # out-projection of block qb-1: 2 chunks per round
                    op0 = out_proj_pieces(qb - 1, d, 0)
                    add(fil, 8, F(op0, 0))
                    add(fil, 9, F(op0, 1))
                    op1 = out_proj_pieces(qb - 1, d, 1)
                    add(fil, 12, F(op1, 0))
                    add(fil, 13, F(op1, 1))
                avs[(d, qb)] = round_ki(d, qb, fil)

        av0l, av1l, flushl = avs.pop((3, NQB - 1))
        flushl()
        round_norm(3, NQB - 1, (av0l, av1l))
        for qt in range(4):
            out_proj(NQB - 1, qt)

    nc.finalize()
    return nc


_NC_CACHE = {}


def make_in_maps(x, y, wq, wk, wv, bq, wo):
    import ml_dtypes
    bf16 = ml_dtypes.bfloat16

    def slab(t):  # [S, D] -> [4, 128, 8, 512] (slab, partition, kc, n)
        return np.ascontiguousarray(
            t.reshape(4, 512, 8, 128).transpose(0, 3, 2, 1)).astype(bf16)

    def wslab(w):  # [D, 512] -> [128, 8, 512]
        return np.ascontiguousarray(
            w.reshape(8, 128, 512).transpose(1, 0, 2)).astype(bf16)

    in_maps = []
    for c in range(8):
        b, hg = c // 2, c % 2
        sl = slice(hg * DS, (hg + 1) * DS)
        in_maps.append({
            "xs": slab(x[b]),
            "ys": slab(y[b]),
            "wq": wslab(wq[:, sl]),
            "wk": wslab(wk[:, sl]),
            "wv": wslab(wv[:, sl]),
            "bq": np.ascontiguousarray(bq[sl].reshape(4, 128).T).astype(np.float32),
            "wo": np.ascontiguousarray(
                wo[sl, :].reshape(4, 128, D).transpose(1, 0, 2)).astype(bf16),
        })
    return in_maps


def kernel(**inputs):
    x = np.asarray(inputs["x"], dtype=np.float32)
    y = np.asarray(inputs["y"], dtype=np.float32)
    wq = np.asarray(inputs["wq"], dtype=np.float32)
    wk = np.asarray(inputs["wk"], dtype=np.float32)
    wv = np.asarray(inputs["wv"], dtype=np.float32)
    wo = np.asarray(inputs["wo"], dtype=np.float32)
    bq = np.asarray(inputs["bq"], dtype=np.float32)
    bv = np.asarray(inputs["bv"], dtype=np.float32)
    bo = np.asarray(inputs["bo"], dtype=np.float32)

    if "nc" not in _NC_CACHE:
        _NC_CACHE["nc"] = build_nc()
    nc = _NC_CACHE["nc"]

    in_maps = make_in_maps(x, y, wq, wk, wv, bq, wo)
    res = run_bass_kernel_spmd(nc, in_maps, list(range(8)))
    outs = [np.asarray(r["out"], dtype=np.float32) for r in res.results]
    full = np.stack([outs[2 * b] + outs[2 * b + 1] for b in range(4)])
    # bk cancels in softmax; bv rides through attention into a constant
    # output offset bv @ wo (softmax rows sum to 1)
    bias = bo + bv @ wo
    return (full + bias[None, None, :]).astype(np.float32)
